# revision 15
# baseline (speedup 1.0000x reference)
"""Trainium2 Bass kernel for nn_DualSPRTLinear: out = x @ (ternary*scales).T

Shapes (hardcoded):
  x       [4, 2048, 4096] fp32   -> tokens T=8192, in-features K=4096
  ternary [4096, 4096]    int8   (out-features O x K), values in {-1,0,1}
  scales  [131072]        fp32   one positive scale per contiguous group of
                                 128 weights (row-major over [O, K]), i.e.
                                 w[o,k] = ternary[o,k] * scales[o*32 + k//128]
  out     [4, 2048, 4096] fp32

Strategy: data-parallel over tokens across 8 NeuronCores (1024 tokens/core).
Each core holds its x-slice transposed ([K, 1024] bf16, SBUF-resident) and
streams the full ternary weight (bf16, transposed to [K, O]) plus
pre-broadcast group scales from HBM, dequantizing on VectorE and matmul'ing
on TensorE (lhsT = x tile [128k x 128t] stationary, rhs = w tile [128k x 512o],
PSUM accumulation over the 32 k-chunks).
"""

import os
import sys

import numpy as np

for _p in ("/opt/trn_rl_repo",):
    if _p not in sys.path and os.path.isdir(_p):
        sys.path.append(_p)

import ml_dtypes

import concourse.bacc as bacc
import concourse.bass as bass
import concourse.mybir as mybir
import concourse.tile as tile
from concourse.bass_utils import run_bass_kernel_spmd

BF16 = ml_dtypes.bfloat16

_AXON_SO = "/opt/axon/libaxon_pjrt.so"


def _ensure_ntff_hook():
    """The agent image's ``antenv`` lacks ``axon_hooks``, so the boot shim
    skips NTFF-hook registration and ``run_bass_kernel_spmd(trace=True)``
    crashes on import. Recreate the module + hook via ctypes on the axon
    PJRT .so (same ABI the boot script uses)."""
    import types

    if "antenv.axon_hooks" in sys.modules:
        return
    import contextlib
    import ctypes

    import antenv

    mod = types.ModuleType("antenv.axon_hooks")
    _state = {"hook": None}
    mod.set_axon_ntff_profile_hook = lambda h: _state.__setitem__("hook", h)
    mod.get_axon_ntff_profile_hook = lambda: _state["hook"]
    sys.modules["antenv.axon_hooks"] = mod
    antenv.axon_hooks = mod

    if not os.path.exists(_AXON_SO):
        return
    lib = ctypes.CDLL(_AXON_SO)
    if not hasattr(lib, "axon_start_nrt_profile"):
        return
    lib.axon_start_nrt_profile.argtypes = [
        ctypes.POINTER(ctypes.c_int64),
        ctypes.c_size_t,
    ]
    lib.axon_start_nrt_profile.restype = ctypes.c_int64
    lib.axon_stop_nrt_profile.argtypes = [ctypes.c_char_p]
    lib.axon_stop_nrt_profile.restype = ctypes.c_int64

    @contextlib.contextmanager
    def _hook(output_dir, device_ids):
        import jax

        jax.devices()
        if device_ids:
            ids = (ctypes.c_int64 * len(device_ids))(*device_ids)
            rc = lib.axon_start_nrt_profile(ids, len(device_ids))
        else:
            rc = lib.axon_start_nrt_profile(None, 0)
        if rc != 0:
            raise RuntimeError(f"axon_start_nrt_profile rc={rc}")
        try:
            yield
        finally:
            n = lib.axon_stop_nrt_profile(str(output_dir).encode())
            print(f"profile: {n} file(s) written to {output_dir}", file=sys.stderr)

    _state["hook"] = _hook

N_CORES = 8
T = 8192          # total tokens
TC = T // N_CORES # tokens per core = 1024
K = 4096          # in-features (contraction)
O = 4096          # out-features
GS = 128          # scale group size == matmul k-chunk
NG = K // GS      # 32 k-chunks
OB = 512          # o-block (matmul free dim / one PSUM bank of fp32)
NJ = O // OB      # 8 o-blocks
GG = 8            # k-chunks per DMA super-tile
NGG = NG // GG    # 4 super-tiles
NM = TC // 128    # 8 token blocks per core


def _build():
    nc = bacc.Bacc(None, target_bir_lowering=False, debug=False)
    xt = nc.dram_tensor("xt", [K, TC], mybir.dt.bfloat16, kind="ExternalInput")
    tt = nc.dram_tensor("tt", [K, O], mybir.dt.int8, kind="ExternalInput")
    sb = nc.dram_tensor("sb", [NG, 128, O], mybir.dt.bfloat16, kind="ExternalInput")
    out = nc.dram_tensor("out", [TC, O], mybir.dt.float32, kind="ExternalOutput")

    xt_r = xt[:].rearrange("(g p) t -> p g t", p=128)   # [128, 32, 1024]
    tt_r = tt[:].rearrange("(g p) o -> p g o", p=128)   # [128, 32, 4096]
    sb_a = sb[:]                                        # [32, 128, 4096]
    out_a = out[:]                                      # [1024, 4096]

    with tile.TileContext(nc) as tc:
        with (
            tc.tile_pool(name="xres", bufs=NG) as xpool,
            tc.tile_pool(name="tern", bufs=3) as tpool,
            tc.tile_pool(name="scal", bufs=3) as spool,
            tc.tile_pool(name="wdeq", bufs=4) as wpool,
            tc.tile_pool(name="ostg", bufs=4) as opool,
            tc.tile_pool(name="warm", bufs=1) as warmpool,
            tc.tile_pool(name="psum", bufs=8, space="PSUM") as ppool,
        ):
            # x slice, transposed+bf16 on host, resident in SBUF for the
            # whole kernel: 32 chunks of [128 k-inner, 1024 tokens] = 8 MiB.
            # Chunks are loaded inside j==0's super-tile loop, paired with
            # the weight super-tile whose matmuls consume them, so the sync
            # HWDGE ring interleaves x with the weight stream instead of
            # starving it (j0 needs x 8 MiB + w 8 MiB ~= its 55 us of PE).
            x_chunks = [None] * NG

            def load_x(g):
                x_g = xpool.tile([128, TC], mybir.dt.bfloat16, name=f"x_{g}", tag="xg")
                nc.sync.dma_start(x_g[:], xt_r[:, g, :])
                x_chunks[g] = x_g

            # PE warm-up: ~7 us of throwaway matmuls so the HAM clock gate
            # is at 2.4 GHz when the real stream starts. Writes land in a
            # psum slot that is released before the real tiles need it.
            warm_sb = warmpool.tile([128, 128], mybir.dt.bfloat16)
            nc.gpsimd.memset(warm_sb[:], 0.0)
            warm_ps = ppool.tile([128, OB], mybir.dt.float32, name="ps_warm", tag="ps")
            for _ in range(24):
                nc.tensor.matmul(
                    warm_ps[:, :128], warm_sb[:], warm_sb[:], start=True, stop=True
                )

            for j in range(NJ):  # output-feature blocks of 512
                osl = slice(j * OB, (j + 1) * OB)
                psum_tiles = [
                    ppool.tile(
                        [128, OB], mybir.dt.float32, name=f"ps_{j}_{m}", tag="ps"
                    )
                    for m in range(NM)
                ]
                # k super-tiles; j0's first ones are small so the first
                # matmul starts as early as possible.
                widths = (2, 6, 8, 8, 8) if j == 0 else (8, 8, 8, 8)
                g0 = 0
                for st, width in enumerate(widths):
                    gsl = slice(g0, g0 + width)
                    # ternary via SWDGE with int8 -> bf16 cast during DMA
                    # (halves its HBM traffic, keeps it off the sync ring)
                    t_tile = tpool.tile(
                        [128, width, OB], mybir.dt.bfloat16,
                        name=f"t_{j}_{st}", tag="t",
                    )
                    nc.gpsimd.dma_start(t_tile[:], tt_r[:, gsl, osl])
                    s_tile = spool.tile(
                        [128, width, OB], mybir.dt.bfloat16,
                        name=f"s_{j}_{st}", tag="s",
                    )
                    nc.sync.dma_start(
                        s_tile[:], sb_a[gsl, :, osl].rearrange("g p o -> p g o")
                    )
                    w_tile = wpool.tile(
                        [128, width, OB], mybir.dt.bfloat16,
                        name=f"w_{j}_{st}", tag="w",
                    )
                    nc.vector.tensor_tensor(
                        w_tile[:], t_tile[:], s_tile[:], mybir.AluOpType.mult
                    )
                    if j == 0:
                        for q in range(width):
                            load_x(g0 + q)
                    for q in range(width):
                        g = g0 + q
                        for m in range(NM):
                            nc.tensor.matmul(
                                psum_tiles[m][:],
                                x_chunks[g][:, m * 128 : (m + 1) * 128],
                                w_tile[:, q, :],
                                start=(g == 0),
                                stop=(g == NG - 1),
                            )
                    g0 += width
                for m in range(NM):
                    o_tile = opool.tile([128, OB], mybir.dt.float32)
                    nc.vector.tensor_copy(o_tile[:], psum_tiles[m][:])
                    nc.scalar.dma_start(
                        out_a[m * 128 : (m + 1) * 128, osl], o_tile[:]
                    )

    nc.compile()
    return nc


_NC = None


def _get_nc():
    global _NC
    if _NC is None:
        _NC = _build()
    return _NC


def _prep_inputs(x, ternary, scales):
    x = np.asarray(x)
    ternary = np.asarray(ternary)
    scales = np.asarray(scales)

    xt = np.ascontiguousarray(x.reshape(T, K).astype(BF16).T)       # [K, T]
    tt = np.ascontiguousarray(ternary.astype(np.int8).T)            # [K, O]
    sr = np.ascontiguousarray(scales.reshape(O, NG).T.astype(BF16)) # [NG, O]
    sb = np.ascontiguousarray(
        np.broadcast_to(sr[:, None, :], (NG, 128, O))
    )  # [NG, 128, O]

    in_maps = []
    for c in range(N_CORES):
        in_maps.append(
            {
                "xt": np.ascontiguousarray(xt[:, c * TC : (c + 1) * TC]),
                "tt": tt,
                "sb": sb,
            }
        )
    return in_maps


def run(x, ternary, scales, trace=False, **trace_kwargs):
    """Run on 8 NeuronCores; returns (out [4,2048,4096] fp32, BassKernelResults)."""
    nc = _get_nc()
    if trace:
        _ensure_ntff_hook()
    in_maps = _prep_inputs(x, ternary, scales)
    res = run_bass_kernel_spmd(
        nc, in_maps, core_ids=list(range(N_CORES)), trace=trace, **trace_kwargs
    )
    parts = [np.asarray(r["out"]) for r in res.results]
    out = np.concatenate(parts, axis=0).reshape(4, 2048, O).astype(np.float32)
    return out, res


def kernel(x, ternary, scales):
    out, _ = run(x, ternary, scales, trace=False)
    return out
